# revision 32
# baseline (speedup 1.0000x reference)
"""Trainium2 Bass kernel for nn_EquiLinear_6708738916908.

Reference computation (BT=32, N_ATOMS=8192, N_CGS=512, KNN=16):
    dist_vec[b, i*K+k, e] = cg_xyz[b, k+1, e] - cg_xyz[b, i, e]
    dx_recon  = einsum('bje,nj->bne', dist_vec, B_param)          # [B, N, 3]
    cg_offset = einsum('bin,bij->bjn', dx_recon, assign_norm)     # [B, C, 3]
    xyz_recon = cg_xyz[:, idx] - cg_offset[:, idx] + dx_recon
    returns (soft_assign, xyz, xyz_recon)   # first two pass through

Key algebraic fold (exact): dist_vec is low-rank in cg_xyz, so
    dx_recon[b,n,e] = sum_c G[n,c] * cg_xyz[b,c,e]
with G[n,c] = (sum_i B[n, i*K + (c-1)] for 1<=c<=K) - sum_k B[n, c*K + k].
G is folded once on the host (float64 accumulate); the device reads G (16MB)
instead of B_param (256MB) and contracts over 512 instead of 8192.

Sharding: data-parallel over batch, 4 batches per core x 8 cores. Each core
reads its assign_norm slice (64MB of bytes, the dominant irreducible HBM
traffic), computes dx for all atoms of its batches and the full cg_offset
for its batches. No collectives. Device returns dx[8192, 12] and
H = cg_xyz - cg_offset [12, 512]; the host does the (tiny) 512-row
gather + add epilogue.

Precision/speed: fp32 matmuls on trn2 cost 4x bf16 (2 LOW/HIGH passes x
2 cyc/col). All big contractions instead run as EXACT bf16 hi/lo pairs:
a = hi + lo with hi = bf16(a), lo = bf16(a - hi) (~17 mantissa bits).
The two stationary halves sit in one [44]-column stationary
([hi | zeros | lo], lo at column 32 so the PSUM fold reads are 32-aligned)
and the hi/lo moving planes accumulate into the same PSUM bank:
    S[0:12]  = hi_s . (hi_m + lo_m)
    S[32:44] = lo_s . (hi_m + lo_m)
    result   = S[0:12] + S[32:44]   (one DVE fold at the end)
which is the exact product of the split operands in fp32 PSUM.

Device pipeline per core:
  A. dxT chunks [12, 512] = cgk-stationary x gt-moving (bf16 hi/lo)
  A'. PE-transpose dxT -> dx tiles [128, 12] fp32 (stationary for B + output)
  B. split dx into the [128, 44] bf16 stationary; per batch accumulate
     S_b += dxs . an_hi + dxs . an_lo over all 64 atom tiles
  C. H = cg_xyzT - S[0:12] - S[32:44] on DVE, DMA out.
"""

import sys

if "/opt/trn_rl_repo" not in sys.path:
    sys.path.insert(0, "/opt/trn_rl_repo")

import ml_dtypes
import numpy as np

import concourse.bass as bass
import concourse.mybir as mybir
import concourse.tile as tile
from concourse.masks import make_identity
from concourse.bass_utils import run_bass_kernel_spmd

BT, N_ATOMS, N_CGS, KNN = 32, 8192, 512, 16
N_CORES = 8
B_LOC = BT // N_CORES          # 4 batches per core
X = 3 * B_LOC                  # 12 fused (batch, xyz-component) columns
XP = 44                        # hi/lo stationary width: [hi(12) pad(20) lo(12)]
LO = 32                        # lo half column offset (32-aligned for PSUM fold)
P = 128
NT = N_ATOMS // P              # 64 atom tiles
KT = N_CGS // P                # 4 cg tiles
GRP = 2                        # atom tiles per assign_norm DMA chunk
NG = NT // GRP                 # 8 groups
CH = min(512, GRP * P)         # matmul-A moving free dim (psum bank limit)
NCH = GRP * P // CH            # 2 dxT chunks per group

BF16 = ml_dtypes.bfloat16


def _split_multi_waits(nc):
    """This toolchain's walrus build encodes a single sem-wait slot per
    instruction and errors on more ("Too many sync wait commands"). Tile's
    scheduler attaches one wait per producer lane, so hoist all but the last
    wait of each instruction onto single-wait NOPs inserted just before it on
    the same engine — identical semantics, since the engine sequencer blocks
    on each wait in program order."""
    ctr = 0
    for f in nc.m.functions:
        for bb in f.blocks:
            old = bb.instructions
            new = []
            changed = False
            for inst in old:
                si = getattr(inst, "sync_info", None)
                if si is not None and si.on_update and len(si.on_update) > 1:
                    raise AssertionError(
                        f"multi-update instruction {inst.name}: unsupported"
                    )
                if si is not None and si.on_wait and len(si.on_wait) > 1:
                    waits = list(si.on_wait)
                    for w in waits[:-1]:
                        nop = mybir.InstNoOp(
                            name=f"splitwait_{ctr}", ins=[], outs=[]
                        )
                        ctr += 1
                        nop.engine = inst.engine
                        nop.sync_info = mybir.SyncInfo(
                            on_wait=[w], on_update=[]
                        )
                        new.append(nop)
                    si.on_wait = waits[-1:]
                    inst.sync_info = si
                    changed = True
                new.append(inst)
            if changed:
                bb.instructions = new
    for f in nc.m.functions:
        for bb in f.blocks:
            for inst in bb.instructions:
                si = getattr(inst, "sync_info", None)
                assert si is None or not si.on_wait or len(si.on_wait) <= 1, (
                    f"multi-wait survived on {inst.name}"
                )


def _build_bass():
    f32 = mybir.dt.float32
    bf = mybir.dt.bfloat16
    nc = bass.Bass("TRN2", target_bir_lowering=False, debug=False)
    # inputs host-packed in the exact SBUF tile layout (partition-major, hi/lo
    # planes interleaved) so every DMA moves 16KB contiguous per partition
    an3 = nc.dram_tensor(
        "an3", [B_LOC, NG, P, 2, GRP, N_CGS], bf, kind="ExternalInput"
    ).ap()
    gt3 = nc.dram_tensor(
        "gt3", [NG, P, 2, KT, GRP * P], bf, kind="ExternalInput"
    ).ap()
    cgks = nc.dram_tensor("cgks", [P, KT, XP], bf, kind="ExternalInput").ap()
    cgt = nc.dram_tensor("cgt", [X, N_CGS], f32, kind="ExternalInput").ap()
    dxout = nc.dram_tensor("dxout", [NG, P, GRP, X], f32, kind="ExternalOutput").ap()
    hout = nc.dram_tensor("hout", [X, N_CGS], f32, kind="ExternalOutput").ap()

    with tile.TileContext(nc) as tc:
        with (
            tc.tile_pool(name="consts", bufs=1) as consts,
            tc.tile_pool(name="gtp", bufs=8) as gtp,
            tc.tile_pool(name="anp", bufs=24) as anp,
            tc.tile_pool(name="dxtp", bufs=2) as dxtp,
            tc.tile_pool(name="dxp", bufs=3) as dxp,
            tc.tile_pool(name="dxsp", bufs=3) as dxsp,
            tc.tile_pool(name="tmps", bufs=3) as tmps,
            tc.tile_pool(name="hp", bufs=2) as hp,
            tc.tile_pool(name="ps_dxt", bufs=2, space="PSUM") as ps_dxt,
            tc.tile_pool(name="ps_tr", bufs=2, space="PSUM") as ps_tr,
            tc.tile_pool(name="ps_cg", bufs=B_LOC, space="PSUM") as ps_cg,
        ):
            # cgks arrives host-striped as [p, kt, xp]: kt slice = [128, 44] lhsT
            cgks_sb = consts.tile([P, KT, XP], bf)
            nc.sync.dma_start(cgks_sb[:], cgks[:])
            cgt_sb = consts.tile([X, N_CGS], f32)
            nc.sync.dma_start(cgt_sb[:], cgt)
            ident = consts.tile([X, X], f32)
            make_identity(nc, ident[:])

            # cg_offset accumulators: rows 0:12 = dx_hi . an, 32:44 = dx_lo . an
            pscg = [
                ps_cg.tile([XP, N_CGS], f32, tag="pscg", name=f"pscg{b}")
                for b in range(B_LOC)
            ]

            for g in range(NG):
                # one 2MB DMA for both gt planes of this group's columns
                gt_t = gtp.tile([P, 2, KT, GRP * P], bf, tag="gt", name=f"gt{g}")
                nc.sync.dma_start(gt_t[:], gt3[g])
                an_ts = []
                for b in range(B_LOC):
                    # one 2MB DMA for both an planes of (batch, group)
                    at = anp.tile(
                        [P, 2, GRP, N_CGS], bf, tag="an", name=f"an{g}_{b}"
                    )
                    nc.sync.dma_start(at[:], an3[b, g])
                    an_ts.append(at)

                # A: dxT[x, n] for this group's atoms, exact bf16 hi/lo pairs
                dxt_sb = dxtp.tile([X, GRP * P], f32, tag="dxt", name=f"dxt{g}")
                for ch in range(NCH):
                    csl = slice(ch * CH, (ch + 1) * CH)
                    pst = ps_dxt.tile([XP, CH], f32, tag="psdxt", name=f"pst{g}_{ch}")
                    n_mm = 0
                    for kt in range(KT):
                        for h in range(2):
                            nc.tensor.matmul(
                                pst[:],
                                cgks_sb[:, kt],
                                gt_t[:, h, kt, csl],
                                start=(n_mm == 0),
                                stop=(n_mm == 2 * KT - 1),
                            )
                            n_mm += 1
                    # fold: dxT = hi-rows + lo-rows (both 32-aligned PSUM reads)
                    nc.vector.tensor_copy(dxt_sb[:, csl], pst[0:X])
                    nc.vector.tensor_tensor(
                        dxt_sb[:, csl],
                        dxt_sb[:, csl],
                        pst[LO : LO + X],
                        mybir.AluOpType.add,
                    )

                # A' + B per atom tile
                dxg = dxp.tile([P, GRP, X], f32, tag="dxg", name=f"dxg{g}")
                for s in range(GRP):
                    it = g * GRP + s
                    ptr = ps_tr.tile([P, X], f32, tag="ptr", name=f"ptr{it}")
                    nc.tensor.transpose(
                        ptr[:], dxt_sb[:, s * P : (s + 1) * P], ident[:]
                    )
                    nc.vector.tensor_copy(dxg[:, s], ptr[:])
                    # build [dx_hi | 0 | dx_lo] bf16 stationary
                    dxs = dxsp.tile([P, XP], bf, tag="dxs", name=f"dxs{it}")
                    nc.vector.memset(dxs[:, X:LO], 0.0)
                    nc.vector.tensor_copy(dxs[:, 0:X], ptr[:])
                    hi32 = tmps.tile([P, X], f32, tag="hi32", name=f"hi32{it}")
                    nc.vector.tensor_copy(hi32[:], dxs[:, 0:X])
                    nc.vector.tensor_tensor(
                        hi32[:], ptr[:], hi32[:], mybir.AluOpType.subtract
                    )
                    nc.vector.tensor_copy(dxs[:, LO : LO + X], hi32[:])
                    for b in range(B_LOC):
                        at = an_ts[b]
                        nc.tensor.matmul(
                            pscg[b][:],
                            dxs[:],
                            at[:, 0, s],
                            start=(it == 0),
                            stop=False,
                        )
                        nc.tensor.matmul(
                            pscg[b][:],
                            dxs[:],
                            at[:, 1, s],
                            start=False,
                            stop=(it == NT - 1),
                        )
                nc.sync.dma_start(dxout[g], dxg[:])

            # C: H = cg_xyzT - cg_offsetT = cgt - S[0:12] - S[32:44]
            for b in range(B_LOC):
                h_b = hp.tile([X, N_CGS], f32, tag="h", name=f"h{b}")
                nc.vector.tensor_tensor(
                    h_b[:], cgt_sb[:], pscg[b][0:X], mybir.AluOpType.subtract
                )
                nc.vector.tensor_tensor(
                    h_b[:], h_b[:], pscg[b][LO : LO + X], mybir.AluOpType.subtract
                )
                nc.sync.dma_start(
                    hout[3 * b : 3 * b + 3, :], h_b[3 * b : 3 * b + 3, :]
                )
    _split_multi_waits(nc)
    return nc


_NC_CACHE = None
_LAST_IN_MAPS = None


def _get_nc():
    global _NC_CACHE
    if _NC_CACHE is None:
        _NC_CACHE = _build_bass()
    return _NC_CACHE


def _fold_g(B_param: np.ndarray, knn: int) -> np.ndarray:
    """G[n,c] such that dx[b,n,e] = sum_c G[n,c] cg_xyz[b,c,e] (exact fold)."""
    Br = B_param.reshape(N_ATOMS, N_CGS, knn)
    Bi = Br.sum(axis=2, dtype=np.float64)          # [n, 512] sum over k
    Bk = Br.sum(axis=1, dtype=np.float64)          # [n, knn] sum over i
    G = -Bi
    G[:, 1 : knn + 1] += Bk
    return G.astype(np.float32)


def _hilo(a: np.ndarray):
    hi = a.astype(BF16)
    lo = (a - hi.astype(np.float32)).astype(BF16)
    return hi, lo


def kernel(xyz, cg_xyz, assign_norm, soft_assign, B_param, assign_idx, knn):
    xyz = np.asarray(xyz, dtype=np.float32)
    cg_xyz = np.asarray(cg_xyz, dtype=np.float32)
    assign_norm = np.asarray(assign_norm, dtype=np.float32)
    soft_assign = np.asarray(soft_assign)
    B_param = np.asarray(B_param, dtype=np.float32)
    idx = np.asarray(assign_idx).astype(np.int64)
    knn = int(knn)

    G = _fold_g(B_param, knn)
    gth, gtl = _hilo(np.ascontiguousarray(G.T))                     # [512, 8192]
    # gt3[g, p, h, kt, n'] = gt_plane_h[kt*128+p, g*1024+n']
    gt3 = np.ascontiguousarray(
        np.stack(
            [
                pl.reshape(KT, P, NG, GRP * P).transpose(2, 1, 0, 3)
                for pl in (gth, gtl)
            ],
            axis=2,
        )
    )                                                               # [NG,P,2,KT,1024]

    in_maps = []
    for c in range(N_CORES):
        cg_c = cg_xyz[c * B_LOC : (c + 1) * B_LOC]                  # [4, 512, 3]
        cgk = cg_c.transpose(1, 0, 2).reshape(N_CGS, X)             # [512, 12]
        ckh, ckl = _hilo(cgk)
        cgks = np.zeros((N_CGS, XP), dtype=BF16)
        cgks[:, 0:X] = ckh
        cgks[:, LO : LO + X] = ckl
        # stripe to the SBUF layout [p, kt, xp] (c = kt*128 + p)
        cgks = np.ascontiguousarray(
            cgks.reshape(KT, P, XP).transpose(1, 0, 2)
        )
        an_h, an_l = _hilo(assign_norm[c * B_LOC : (c + 1) * B_LOC])
        # an3[b, g, p, h, s, j] = an_plane_h[b, g*1024 + s*128 + p, j]
        an3 = np.ascontiguousarray(
            np.stack(
                [
                    pl.reshape(B_LOC, NG, GRP, P, N_CGS).transpose(0, 1, 3, 2, 4)
                    for pl in (an_h, an_l)
                ],
                axis=3,
            )
        )                                                           # [B,NG,P,2,GRP,J]
        in_maps.append(
            {
                "an3": an3,
                "gt3": gt3,
                "cgks": cgks,
                "cgt": np.ascontiguousarray(
                    cg_c.transpose(0, 2, 1).reshape(X, N_CGS)
                ),
            }
        )

    global _LAST_IN_MAPS
    _LAST_IN_MAPS = in_maps
    res = run_bass_kernel_spmd(_get_nc(), in_maps, core_ids=list(range(N_CORES)))

    xyz_recon = np.empty((BT, N_ATOMS, 3), dtype=np.float32)
    for c in range(N_CORES):
        out = res.results[c]
        # dxout[g, p, s, x]: atom n = g*GRP*128 + s*128 + p, column x = 3*b + e
        dx = (
            out["dxout"].transpose(0, 2, 1, 3).reshape(N_ATOMS, B_LOC, 3)
        )                                                            # [n, b, e]
        H = out["hout"].reshape(B_LOC, 3, N_CGS)                     # [b, e, j]
        # xyz_recon[b, n, e] = dx[n, b, e] + H[b, e, idx[n]]
        xyz_recon[c * B_LOC : (c + 1) * B_LOC] = dx.transpose(1, 0, 2) + H[
            :, :, idx
        ].transpose(0, 2, 1)

    return (soft_assign, xyz, xyz_recon)


# revision 33
# speedup vs baseline: 1.2429x; 1.2429x over previous
"""Trainium2 Bass kernel for nn_EquiLinear_6708738916908.

Reference computation (BT=32, N_ATOMS=8192, N_CGS=512, KNN=16):
    dist_vec[b, i*K+k, e] = cg_xyz[b, k+1, e] - cg_xyz[b, i, e]
    dx_recon  = einsum('bje,nj->bne', dist_vec, B_param)          # [B, N, 3]
    cg_offset = einsum('bin,bij->bjn', dx_recon, assign_norm)     # [B, C, 3]
    xyz_recon = cg_xyz[:, idx] - cg_offset[:, idx] + dx_recon
    returns (soft_assign, xyz, xyz_recon)   # first two pass through

Key algebraic fold (exact): dist_vec is low-rank in cg_xyz, so
    dx_recon[b,n,e] = sum_c G[n,c] * cg_xyz[b,c,e]
with G[n,c] = (sum_i B[n, i*K + (c-1)] for 1<=c<=K) - sum_k B[n, c*K + k].
G is folded once on the host (float64 accumulate); the device reads G (16MB)
instead of B_param (256MB) and contracts over 512 instead of 8192.

Sharding: data-parallel over batch, 4 batches per core x 8 cores. Each core
reads its assign_norm slice (64MB of bytes, the dominant irreducible HBM
traffic), computes dx for all atoms of its batches and the full cg_offset
for its batches. No collectives. Device returns dx[8192, 12] and
H = cg_xyz - cg_offset [12, 512]; the host does the (tiny) 512-row
gather + add epilogue.

Precision/speed: fp32 matmuls on trn2 cost 4x bf16 (2 LOW/HIGH passes x
2 cyc/col). All big contractions instead run as EXACT bf16 hi/lo pairs:
a = hi + lo with hi = bf16(a), lo = bf16(a - hi) (~17 mantissa bits).
The two stationary halves sit in one [44]-column stationary
([hi | zeros | lo], lo at column 32 so the PSUM fold reads are 32-aligned)
and the hi/lo moving planes accumulate into the same PSUM bank:
    S[0:12]  = hi_s . (hi_m + lo_m)
    S[32:44] = lo_s . (hi_m + lo_m)
    result   = S[0:12] + S[32:44]   (one DVE fold at the end)
which is the exact product of the split operands in fp32 PSUM.

Device pipeline per core:
  A. dxT chunks [12, 512] = cgk-stationary x gt-moving (bf16 hi/lo)
  A'. PE-transpose dxT -> dx tiles [128, 12] fp32 (stationary for B + output)
  B. split dx into the [128, 44] bf16 stationary; per batch accumulate
     S_b += dxs . an_hi + dxs . an_lo over all 64 atom tiles
  C. H = cg_xyzT - S[0:12] - S[32:44] on DVE, DMA out.
"""

import sys

if "/opt/trn_rl_repo" not in sys.path:
    sys.path.insert(0, "/opt/trn_rl_repo")

import ml_dtypes
import numpy as np

import concourse.bass as bass
import concourse.mybir as mybir
import concourse.tile as tile
from concourse.masks import make_identity
from concourse.bass_utils import run_bass_kernel_spmd

BT, N_ATOMS, N_CGS, KNN = 32, 8192, 512, 16
N_CORES = 8
B_LOC = BT // N_CORES          # 4 batches per core
X = 3 * B_LOC                  # 12 fused (batch, xyz-component) columns
XP = 44                        # hi/lo stationary width: [hi(12) pad(20) lo(12)]
LO = 32                        # lo half column offset (32-aligned for PSUM fold)
P = 128
NT = N_ATOMS // P              # 64 atom tiles
KT = N_CGS // P                # 4 cg tiles
GRP = 4                        # atom tiles per assign_norm DMA chunk
NG = NT // GRP                 # 8 groups
CH = min(512, GRP * P)         # matmul-A moving free dim (psum bank limit)
NCH = GRP * P // CH            # 2 dxT chunks per group

BF16 = ml_dtypes.bfloat16


def _split_multi_waits(nc):
    """This toolchain's walrus build encodes a single sem-wait slot per
    instruction and errors on more ("Too many sync wait commands"). Tile's
    scheduler attaches one wait per producer lane, so hoist all but the last
    wait of each instruction onto single-wait NOPs inserted just before it on
    the same engine — identical semantics, since the engine sequencer blocks
    on each wait in program order."""
    ctr = 0
    for f in nc.m.functions:
        for bb in f.blocks:
            old = bb.instructions
            new = []
            changed = False
            for inst in old:
                si = getattr(inst, "sync_info", None)
                if si is not None and si.on_update and len(si.on_update) > 1:
                    raise AssertionError(
                        f"multi-update instruction {inst.name}: unsupported"
                    )
                if si is not None and si.on_wait and len(si.on_wait) > 1:
                    waits = list(si.on_wait)
                    for w in waits[:-1]:
                        nop = mybir.InstNoOp(
                            name=f"splitwait_{ctr}", ins=[], outs=[]
                        )
                        ctr += 1
                        nop.engine = inst.engine
                        nop.sync_info = mybir.SyncInfo(
                            on_wait=[w], on_update=[]
                        )
                        new.append(nop)
                    si.on_wait = waits[-1:]
                    inst.sync_info = si
                    changed = True
                new.append(inst)
            if changed:
                bb.instructions = new
    for f in nc.m.functions:
        for bb in f.blocks:
            for inst in bb.instructions:
                si = getattr(inst, "sync_info", None)
                assert si is None or not si.on_wait or len(si.on_wait) <= 1, (
                    f"multi-wait survived on {inst.name}"
                )


def _build_bass():
    f32 = mybir.dt.float32
    bf = mybir.dt.bfloat16
    nc = bass.Bass("TRN2", target_bir_lowering=False, debug=False)
    # inputs host-packed in the exact SBUF tile layout (partition-major, hi/lo
    # planes interleaved) so every DMA moves 16KB contiguous per partition
    an3 = nc.dram_tensor(
        "an3", [B_LOC, NG, P, 2, GRP, N_CGS], bf, kind="ExternalInput"
    ).ap()
    gt3 = nc.dram_tensor(
        "gt3", [NG, P, 2, KT, GRP * P], bf, kind="ExternalInput"
    ).ap()
    cgks = nc.dram_tensor("cgks", [P, KT, XP], bf, kind="ExternalInput").ap()
    cgt = nc.dram_tensor("cgt", [X, N_CGS], f32, kind="ExternalInput").ap()
    dxout = nc.dram_tensor("dxout", [NG, P, GRP, X], f32, kind="ExternalOutput").ap()
    hout = nc.dram_tensor("hout", [X, N_CGS], f32, kind="ExternalOutput").ap()

    with tile.TileContext(nc) as tc:
        with (
            tc.tile_pool(name="consts", bufs=1) as consts,
            tc.tile_pool(name="gtp", bufs=5) as gtp,
            tc.tile_pool(name="anp", bufs=12) as anp,
            tc.tile_pool(name="dxtp", bufs=2) as dxtp,
            tc.tile_pool(name="dxp", bufs=3) as dxp,
            tc.tile_pool(name="dxsp", bufs=3) as dxsp,
            tc.tile_pool(name="tmps", bufs=3) as tmps,
            tc.tile_pool(name="hp", bufs=2) as hp,
            tc.tile_pool(name="ps_dxt", bufs=2, space="PSUM") as ps_dxt,
            tc.tile_pool(name="ps_tr", bufs=2, space="PSUM") as ps_tr,
            tc.tile_pool(name="ps_cg", bufs=B_LOC, space="PSUM") as ps_cg,
        ):
            # cgks arrives host-striped as [p, kt, xp]: kt slice = [128, 44] lhsT
            cgks_sb = consts.tile([P, KT, XP], bf)
            nc.sync.dma_start(cgks_sb[:], cgks[:])
            cgt_sb = consts.tile([X, N_CGS], f32)
            nc.sync.dma_start(cgt_sb[:], cgt)
            ident = consts.tile([X, X], f32)
            make_identity(nc, ident[:])

            # cg_offset accumulators: rows 0:12 = dx_hi . an, 32:44 = dx_lo . an
            pscg = [
                ps_cg.tile([XP, N_CGS], f32, tag="pscg", name=f"pscg{b}")
                for b in range(B_LOC)
            ]

            for g in range(NG):
                # one 2MB DMA for both gt planes of this group's columns
                gt_t = gtp.tile([P, 2, KT, GRP * P], bf, tag="gt", name=f"gt{g}")
                nc.sync.dma_start(gt_t[:], gt3[g])
                an_ts = []
                for b in range(B_LOC):
                    # one 2MB DMA for both an planes of (batch, group)
                    at = anp.tile(
                        [P, 2, GRP, N_CGS], bf, tag="an", name=f"an{g}_{b}"
                    )
                    nc.sync.dma_start(at[:], an3[b, g])
                    an_ts.append(at)

                # A: dxT[x, n] for this group's atoms, exact bf16 hi/lo pairs
                dxt_sb = dxtp.tile([X, GRP * P], f32, tag="dxt", name=f"dxt{g}")
                for ch in range(NCH):
                    csl = slice(ch * CH, (ch + 1) * CH)
                    pst = ps_dxt.tile([XP, CH], f32, tag="psdxt", name=f"pst{g}_{ch}")
                    n_mm = 0
                    for kt in range(KT):
                        for h in range(2):
                            nc.tensor.matmul(
                                pst[:],
                                cgks_sb[:, kt],
                                gt_t[:, h, kt, csl],
                                start=(n_mm == 0),
                                stop=(n_mm == 2 * KT - 1),
                            )
                            n_mm += 1
                    # fold: dxT = hi-rows + lo-rows (both 32-aligned PSUM reads)
                    nc.vector.tensor_copy(dxt_sb[:, csl], pst[0:X])
                    nc.vector.tensor_tensor(
                        dxt_sb[:, csl],
                        dxt_sb[:, csl],
                        pst[LO : LO + X],
                        mybir.AluOpType.add,
                    )

                # A' + B per atom tile
                dxg = dxp.tile([P, GRP, X], f32, tag="dxg", name=f"dxg{g}")
                for s in range(GRP):
                    it = g * GRP + s
                    ptr = ps_tr.tile([P, X], f32, tag="ptr", name=f"ptr{it}")
                    nc.tensor.transpose(
                        ptr[:], dxt_sb[:, s * P : (s + 1) * P], ident[:]
                    )
                    nc.vector.tensor_copy(dxg[:, s], ptr[:])
                    # build [dx_hi | 0 | dx_lo] bf16 stationary
                    dxs = dxsp.tile([P, XP], bf, tag="dxs", name=f"dxs{it}")
                    nc.vector.memset(dxs[:, X:LO], 0.0)
                    nc.vector.tensor_copy(dxs[:, 0:X], ptr[:])
                    hi32 = tmps.tile([P, X], f32, tag="hi32", name=f"hi32{it}")
                    nc.vector.tensor_copy(hi32[:], dxs[:, 0:X])
                    nc.vector.tensor_tensor(
                        hi32[:], ptr[:], hi32[:], mybir.AluOpType.subtract
                    )
                    nc.vector.tensor_copy(dxs[:, LO : LO + X], hi32[:])
                    for b in range(B_LOC):
                        at = an_ts[b]
                        nc.tensor.matmul(
                            pscg[b][:],
                            dxs[:],
                            at[:, 0, s],
                            start=(it == 0),
                            stop=False,
                        )
                        nc.tensor.matmul(
                            pscg[b][:],
                            dxs[:],
                            at[:, 1, s],
                            start=False,
                            stop=(it == NT - 1),
                        )
                nc.sync.dma_start(dxout[g], dxg[:])

            # C: H = cg_xyzT - cg_offsetT = cgt - S[0:12] - S[32:44]
            for b in range(B_LOC):
                h_b = hp.tile([X, N_CGS], f32, tag="h", name=f"h{b}")
                nc.vector.tensor_tensor(
                    h_b[:], cgt_sb[:], pscg[b][0:X], mybir.AluOpType.subtract
                )
                nc.vector.tensor_tensor(
                    h_b[:], h_b[:], pscg[b][LO : LO + X], mybir.AluOpType.subtract
                )
                nc.sync.dma_start(
                    hout[3 * b : 3 * b + 3, :], h_b[3 * b : 3 * b + 3, :]
                )
    _split_multi_waits(nc)
    return nc


_NC_CACHE = None
_LAST_IN_MAPS = None


def _get_nc():
    global _NC_CACHE
    if _NC_CACHE is None:
        _NC_CACHE = _build_bass()
    return _NC_CACHE


def _fold_g(B_param: np.ndarray, knn: int) -> np.ndarray:
    """G[n,c] such that dx[b,n,e] = sum_c G[n,c] cg_xyz[b,c,e] (exact fold)."""
    Br = B_param.reshape(N_ATOMS, N_CGS, knn)
    Bi = Br.sum(axis=2, dtype=np.float64)          # [n, 512] sum over k
    Bk = Br.sum(axis=1, dtype=np.float64)          # [n, knn] sum over i
    G = -Bi
    G[:, 1 : knn + 1] += Bk
    return G.astype(np.float32)


def _hilo(a: np.ndarray):
    hi = a.astype(BF16)
    lo = (a - hi.astype(np.float32)).astype(BF16)
    return hi, lo


def kernel(xyz, cg_xyz, assign_norm, soft_assign, B_param, assign_idx, knn):
    xyz = np.asarray(xyz, dtype=np.float32)
    cg_xyz = np.asarray(cg_xyz, dtype=np.float32)
    assign_norm = np.asarray(assign_norm, dtype=np.float32)
    soft_assign = np.asarray(soft_assign)
    B_param = np.asarray(B_param, dtype=np.float32)
    idx = np.asarray(assign_idx).astype(np.int64)
    knn = int(knn)

    G = _fold_g(B_param, knn)
    gth, gtl = _hilo(np.ascontiguousarray(G.T))                     # [512, 8192]
    # gt3[g, p, h, kt, n'] = gt_plane_h[kt*128+p, g*1024+n']
    gt3 = np.ascontiguousarray(
        np.stack(
            [
                pl.reshape(KT, P, NG, GRP * P).transpose(2, 1, 0, 3)
                for pl in (gth, gtl)
            ],
            axis=2,
        )
    )                                                               # [NG,P,2,KT,1024]

    in_maps = []
    for c in range(N_CORES):
        cg_c = cg_xyz[c * B_LOC : (c + 1) * B_LOC]                  # [4, 512, 3]
        cgk = cg_c.transpose(1, 0, 2).reshape(N_CGS, X)             # [512, 12]
        ckh, ckl = _hilo(cgk)
        cgks = np.zeros((N_CGS, XP), dtype=BF16)
        cgks[:, 0:X] = ckh
        cgks[:, LO : LO + X] = ckl
        # stripe to the SBUF layout [p, kt, xp] (c = kt*128 + p)
        cgks = np.ascontiguousarray(
            cgks.reshape(KT, P, XP).transpose(1, 0, 2)
        )
        an_h, an_l = _hilo(assign_norm[c * B_LOC : (c + 1) * B_LOC])
        # an3[b, g, p, h, s, j] = an_plane_h[b, g*1024 + s*128 + p, j]
        an3 = np.ascontiguousarray(
            np.stack(
                [
                    pl.reshape(B_LOC, NG, GRP, P, N_CGS).transpose(0, 1, 3, 2, 4)
                    for pl in (an_h, an_l)
                ],
                axis=3,
            )
        )                                                           # [B,NG,P,2,GRP,J]
        in_maps.append(
            {
                "an3": an3,
                "gt3": gt3,
                "cgks": cgks,
                "cgt": np.ascontiguousarray(
                    cg_c.transpose(0, 2, 1).reshape(X, N_CGS)
                ),
            }
        )

    global _LAST_IN_MAPS
    _LAST_IN_MAPS = in_maps
    res = run_bass_kernel_spmd(_get_nc(), in_maps, core_ids=list(range(N_CORES)))

    xyz_recon = np.empty((BT, N_ATOMS, 3), dtype=np.float32)
    for c in range(N_CORES):
        out = res.results[c]
        # dxout[g, p, s, x]: atom n = g*GRP*128 + s*128 + p, column x = 3*b + e
        dx = (
            out["dxout"].transpose(0, 2, 1, 3).reshape(N_ATOMS, B_LOC, 3)
        )                                                            # [n, b, e]
        H = out["hout"].reshape(B_LOC, 3, N_CGS)                     # [b, e, j]
        # xyz_recon[b, n, e] = dx[n, b, e] + H[b, e, idx[n]]
        xyz_recon[c * B_LOC : (c + 1) * B_LOC] = dx.transpose(1, 0, 2) + H[
            :, :, idx
        ].transpose(0, 2, 1)

    return (soft_assign, xyz, xyz_recon)
